# revision 20
# baseline (speedup 1.0000x reference)
"""MetaLSTMCell Trainium2 kernel.

Data-parallel over 8 NeuronCores: batch 8192 -> 1024 rows/core, weights
replicated. Per core:
  - fp32 -> fp16 casts happen inside SWDGE DMA loads; fp16 -> fp32 on the
    output stores.
  - Feature-major copies of input/main_h/meta_h and all weights produced by
    ~15 batched DMA XBAR transposes (interleaved chunk layouts + strided
    APs). The three dz hypernet weights + main bias are staged as stacked
    columns so ONE transpose yields the [97, 2048] stacked operand whose
    partition rows line up with the stacked z vector for row-packed K=32
    matmuls.
  - Matmuls fp16, fp32 PSUM accumulation, N=512 moving tiles.
  - Gate assembly pre = Mzi*A + MzH*B + C(+bias) spread across DVE/ACT/POOL.
"""

import sys

import numpy as np

if "/opt/trn_rl_repo" not in sys.path:
    sys.path.insert(0, "/opt/trn_rl_repo")

I_, H_, HH_, E_ = 512, 512, 128, 32
G4H, G4HH = 4 * H_, 4 * HH_  # 2048, 512
N_CORES = 8
B_FULL = 8192
B_ = B_FULL // N_CORES  # 1024 rows per core
NBT = B_ // 128  # 8 batch tiles per core

_BATCH_KEYS = ("input", "main_h", "main_c", "meta_h", "meta_c")

_CACHE = {}


def _build(reps=1, mode="full"):
    import concourse.bacc as bacc
    import concourse.mybir as mybir
    import concourse.tile as tile

    f16 = mybir.dt.float16
    f32 = mybir.dt.float32
    ACT = mybir.ActivationFunctionType

    nc = bacc.Bacc(
        "TRN2", target_bir_lowering=False, debug=False, enable_asserts=False
    )

    def gload(out, in_):
        if mode != "comp":
            nc.gpsimd.dma_start(out, in_)
        else:
            nc.gpsimd.memset(out, 0.25)

    # ---- DRAM I/O ------------------------------------------------------
    x_d = nc.dram_tensor("input", [B_, I_], f32, kind="ExternalInput").ap()
    mh_d = nc.dram_tensor("main_h", [B_, H_], f32, kind="ExternalInput").ap()
    mc_d = nc.dram_tensor("main_c", [B_, H_], f32, kind="ExternalInput").ap()
    zh_d = nc.dram_tensor("meta_h", [B_, HH_], f32, kind="ExternalInput").ap()
    zc_d = nc.dram_tensor("meta_c", [B_, HH_], f32, kind="ExternalInput").ap()

    wiH_d = nc.dram_tensor("weight_iH", [G4H, I_], f32, kind="ExternalInput").ap()
    wHH_d = nc.dram_tensor("weight_HH", [G4H, H_], f32, kind="ExternalInput").ap()
    wih_d = nc.dram_tensor("weight_ih", [G4HH, I_ + H_], f32, kind="ExternalInput").ap()
    whh_d = nc.dram_tensor("weight_hh", [G4HH, HH_], f32, kind="ExternalInput").ap()
    whzi_d = nc.dram_tensor("weight_hzi", [E_, HH_], f32, kind="ExternalInput").ap()
    whzH_d = nc.dram_tensor("weight_hzH", [E_, HH_], f32, kind="ExternalInput").ap()
    whzb_d = nc.dram_tensor("weight_hzb", [E_, HH_], f32, kind="ExternalInput").ap()
    wdzi_d = nc.dram_tensor("weight_dziH", [G4H, E_], f32, kind="ExternalInput").ap()
    wdzH_d = nc.dram_tensor("weight_dzHH", [G4H, E_], f32, kind="ExternalInput").ap()
    wbz_d = nc.dram_tensor("weight_bzH", [G4H, E_], f32, kind="ExternalInput").ap()
    bias_i_d = nc.dram_tensor("bias_i", [E_, 1], f32, kind="ExternalInput").ap()
    bias_H_d = nc.dram_tensor("bias_H", [E_, 1], f32, kind="ExternalInput").ap()
    bias_d = nc.dram_tensor("bias", [1, G4H], f32, kind="ExternalInput").ap()
    bias_hy_d = nc.dram_tensor("bias_hyper", [1, G4HH], f32, kind="ExternalInput").ap()

    mhn_d = nc.dram_tensor("main_h_new", [B_, H_], f32, kind="ExternalOutput").ap()
    mcn_d = nc.dram_tensor("main_c_new", [B_, H_], f32, kind="ExternalOutput").ap()
    zhn_d = nc.dram_tensor("meta_h_new", [B_, HH_], f32, kind="ExternalOutput").ap()
    zcn_d = nc.dram_tensor("meta_c_new", [B_, HH_], f32, kind="ExternalOutput").ap()

    with tile.TileContext(nc) as tc:
        with (
            tc.tile_pool(name="wpool", bufs=1) as wpool,
            tc.tile_pool(name="apool", bufs=1) as apool,
            tc.tile_pool(name="epool", bufs=3) as epool,
            tc.tile_pool(name="gpool", bufs=2) as gpool,
            tc.tile_pool(name="opool", bufs=1) as opool,
        ):
            for rep_ in range(reps):
                # ============ P0a: activations needed for matmuls =========
                zh_all = apool.tile([128, NBT, HH_], f16)
                gload(zh_all, zh_d.rearrange("(c p) f -> p c f", p=128))
                in_all = apool.tile([128, NBT, I_], f16)
                gload(in_all, x_d.rearrange("(c p) f -> p c f", p=128))
                mh_all = apool.tile([128, NBT, H_], f16)
                gload(mh_all, mh_d.rearrange("(c p) f -> p c f", p=128))

                # transposes of those (one DMA each)
                meta_hT = wpool.tile([128, B_], f16)
                nc.sync.dma_start_transpose(
                    meta_hT.rearrange("p (c q) -> p c q", q=128), zh_all[:, :, :]
                )
                inT_all = wpool.tile([128, 4 * NBT, 128], f16)
                nc.sync.dma_start_transpose(inT_all, in_all[:, :, :])
                mhT_all = wpool.tile([128, 4 * NBT, 128], f16)
                nc.sync.dma_start_transpose(mhT_all, mh_all[:, :, :])

                # ============ P0b: z/meta path weights ====================
                # small z weights [32, 128] -> [128, 32]
                shz = []
                for j, wd in enumerate((whzi_d, whzH_d, whzb_d)):
                    s = wpool.tile([E_, HH_], f16, tag=f"shz{j}", name=f"shz{j}")
                    gload(s, wd)
                    shz.append(s)
                whzT = []
                for j in range(3):
                    t = wpool.tile([128, E_], f16, tag=f"whzT{j}", name=f"whzT{j}")
                    nc.sync.dma_start_transpose(t, shz[j])
                    whzT.append(t)

                bias_i_sb = wpool.tile([E_, 1], f32)
                if mode != "comp":
                    nc.sync.dma_start(bias_i_sb, bias_i_d)
                else:
                    nc.vector.memset(bias_i_sb, 0.1)
                bias_H_sb = wpool.tile([E_, 1], f32)
                if mode != "comp":
                    nc.sync.dma_start(bias_H_sb, bias_H_d)
                else:
                    nc.vector.memset(bias_H_sb, 0.1)
                bias_hy_sb = wpool.tile([1, G4HH], f16)
                gload(bias_hy_sb, bias_hy_d)
                ones1 = wpool.tile([1, 128], f16)
                nc.vector.memset(ones1, 1.0)

                # dz weights + main bias staged as stacked columns:
                # sdz3[:, c, 0:32]=Wdzi | 32:64=WdzH | 64:96=Wbz | 96=bias
                sdz3_32 = wpool.tile([128, 16, 128], f32)
                nc.vector.memset(sdz3_32, 0.0)
                if mode != "comp":
                    nc.sync.dma_start(
                        sdz3_32[:, :, 0:E_],
                        wdzi_d.rearrange("(c p) e -> p c e", p=128),
                    )
                    nc.sync.dma_start(
                        sdz3_32[:, :, E_ : 2 * E_],
                        wdzH_d.rearrange("(c p) e -> p c e", p=128),
                    )
                    nc.sync.dma_start(
                        sdz3_32[:, :, 2 * E_ : 3 * E_],
                        wbz_d.rearrange("(c p) e -> p c e", p=128),
                    )
                    nc.sync.dma_start(
                        sdz3_32[:, :, 3 * E_ : 3 * E_ + 1],
                        bias_d.rearrange("x (c p) -> p c x", p=128),
                    )
                sdz3 = wpool.tile([128, 16, 128], f16)
                nc.vector.tensor_copy(sdz3, sdz3_32)
                # one transpose -> [128, 16, 128]: rows 0:32 Wdzi^T, 32:64
                # WdzH^T, 64:96 Wbz^T, row 96 bias; cols (c, q): g = 128c+q
                wdzT = wpool.tile([128, 16, 128], f16)
                nc.sync.dma_start_transpose(wdzT, sdz3[:, :, :])

                # meta weights
                sih = wpool.tile(
                    [128, 4, I_ + H_], f16, tag="wstage", name="sih", bufs=2
                )
                gload(sih, wih_d.rearrange("(c p) f -> p c f", p=128))
                wihT = wpool.tile([128, 32, 128], f16)
                nc.sync.dma_start_transpose(wihT, sih[:, :, :])
                shh = wpool.tile([128, 4, HH_], f16, tag="wstage", name="shh", bufs=2)
                gload(shh, whh_d.rearrange("(c p) f -> p c f", p=128))
                whhT = wpool.tile([128, 4, 128], f16)
                nc.sync.dma_start_transpose(whhT, shh[:, :, :])

                # ============ P0c: big main weights =======================
                wiHT = [
                    wpool.tile([128, 16, 128], f16, tag=f"wiHT{j}", name=f"wiHT{j}")
                    for j in range(4)
                ]
                wHHT = [
                    wpool.tile([128, 16, 128], f16, tag=f"wHHT{j}", name=f"wHHT{j}")
                    for j in range(4)
                ]
                for h in range(2):
                    s = wpool.tile(
                        [128, 8, I_], f16, tag="wstage", name=f"siH{h}", bufs=2
                    )
                    gload(
                        s,
                        wiH_d[1024 * h : 1024 * (h + 1), :].rearrange(
                            "(c p) f -> p c f", p=128
                        ),
                    )
                    for i in range(2):
                        nc.sync.dma_start_transpose(
                            wiHT[2 * h + i], s[:, 4 * i : 4 * i + 4, :]
                        )
                    s = wpool.tile(
                        [128, 8, H_], f16, tag="wstage", name=f"sHH{h}", bufs=2
                    )
                    gload(
                        s,
                        wHH_d[1024 * h : 1024 * (h + 1), :].rearrange(
                            "(c p) f -> p c f", p=128
                        ),
                    )
                    for i in range(2):
                        nc.sync.dma_start_transpose(
                            wHHT[2 * h + i], s[:, 4 * i : 4 * i + 4, :]
                        )

                # ============ P0d: elementwise-only data ==================
                zc_all = apool.tile([128, NBT, HH_], f16)
                gload(zc_all, zc_d.rearrange("(c p) f -> p c f", p=128))
                mc_all = apool.tile([128, NBT, H_], f16)
                gload(mc_all, mc_d.rearrange("(c p) f -> p c f", p=128))

                # fp16 output accumulation tiles (one cast store at the end)
                out_c = opool.tile([128, NBT, H_], f16)
                out_h = opool.tile([128, NBT, H_], f16)
                out_zc = opool.tile([128, NBT, HH_], f16)
                out_zh = opool.tile([128, NBT, HH_], f16)

                # ============ P2: hypernet z path =========================
                with tc.tile_pool(name="zpool", bufs=1, space="PSUM") as zpool:
                    z_ps = []
                    for j in range(3):
                        zp = zpool.tile([E_, B_], f32, tag=f"z{j}", name=f"z{j}")
                        for h in range(B_ // 512):
                            nc.tensor.matmul(
                                zp[:, 512 * h : 512 * (h + 1)],
                                whzT[j],
                                meta_hT[:, 512 * h : 512 * (h + 1)],
                                start=True,
                                stop=True,
                            )
                        z_ps.append(zp)

                    # zstack rows: 0:32 zi | 32:64 zH | 64:96 zb | 96 ones
                    zstack = wpool.tile([97, B_], f16)
                    nc.vector.tensor_scalar_add(zstack[0:E_, :], z_ps[0], bias_i_sb)
                    nc.vector.tensor_scalar_add(
                        zstack[E_ : 2 * E_, :], z_ps[1], bias_H_sb
                    )
                    nc.scalar.activation(zstack[2 * E_ : 3 * E_, :], z_ps[2], ACT.Copy)
                    nc.vector.memset(zstack[3 * E_ : 3 * E_ + 1, :], 1.0)

                # ============ P3: gate-major compute ======================
                pspool_cm = tc.tile_pool(name="pspool", bufs=1, space="PSUM")
                pspool = pspool_cm.__enter__()

                def lin(bt, k):
                    return inT_all[:, 4 * bt + k, :]

                def lmh(bt, k):
                    return mhT_all[:, 4 * bt + k, :]

                # ---- meta (hyper) LSTM: early PE work while weights load --
                for bt in range(NBT):
                    btsl = slice(bt * 128, (bt + 1) * 128)
                    ps_meta = pspool.tile([128, G4HH], f32, tag="meta", bufs=1)
                    for k in range(4):
                        nc.tensor.matmul(
                            ps_meta, lin(bt, k), wihT[:, k::8, :],
                            start=(k == 0), stop=False,
                        )
                    for k in range(4):
                        nc.tensor.matmul(
                            ps_meta, lmh(bt, k), wihT[:, 4 + k :: 8, :],
                            start=False, stop=False,
                        )
                    nc.tensor.matmul(
                        ps_meta, meta_hT[:, btsl], whhT, start=False, stop=False
                    )
                    nc.tensor.matmul(
                        ps_meta, ones1, bias_hy_sb, start=False, stop=True
                    )

                    smi = epool.tile([128, HH_], f16, tag="smi")
                    nc.scalar.activation(smi, ps_meta[:, 0:128], ACT.Sigmoid)
                    smf = epool.tile([128, HH_], f16, tag="smf")
                    nc.scalar.activation(smf, ps_meta[:, 128:256], ACT.Sigmoid)
                    tmg = epool.tile([128, HH_], f16, tag="tmg")
                    nc.scalar.activation(tmg, ps_meta[:, 256:384], ACT.Tanh)
                    smo = epool.tile([128, HH_], f16, tag="smo")
                    nc.scalar.activation(smo, ps_meta[:, 384:512], ACT.Sigmoid)

                    v1 = epool.tile([128, HH_], f16, tag="v1")
                    nc.gpsimd.tensor_mul(v1, smf, zc_all[:, bt, :])
                    v2 = epool.tile([128, HH_], f16, tag="v2")
                    nc.gpsimd.tensor_mul(v2, smi, tmg)
                    zc_new = out_zc[:, bt, :]
                    nc.vector.tensor_add(zc_new, v1, v2)
                    tmc = epool.tile([128, HH_], f16, tag="tmc")
                    nc.scalar.activation(tmc, zc_new, ACT.Tanh)
                    nc.gpsimd.tensor_mul(out_zh[:, bt, :], smo, tmc)

                # ---- main LSTM gates, nt-major, combine fused in ----
                # order: f(1) -> i(0) -> g(2) -> o(3)
                g0_all = opool.tile([128, NBT, 512], f16)
                u1_all = opool.tile([128, NBT, 512], f16)
                tc_all = opool.tile([128, NBT, 512], f16)
                for nt in [1, 0, 2, 3]:
                    for bt in range(NBT):
                        btsl = slice(bt * 128, (bt + 1) * 128)
                        psMM3 = pspool.tile([128, 1536], f32, tag="MM3", bufs=1)
                        nc.tensor.matmul(
                            psMM3[:, 0:512],
                            zstack[0:E_, btsl],
                            wdzT[0:E_, 4 * nt : 4 * nt + 4, :],
                            start=True, stop=True,
                        )
                        nc.tensor.matmul(
                            psMM3[:, 512:1024],
                            zstack[E_ : 2 * E_, btsl],
                            wdzT[E_ : 2 * E_, 4 * nt : 4 * nt + 4, :],
                            start=True, stop=True,
                        )
                        nc.tensor.matmul(
                            psMM3[:, 1024:1536],
                            zstack[2 * E_ : 3 * E_ + 1, btsl],
                            wdzT[2 * E_ : 3 * E_ + 1, 4 * nt : 4 * nt + 4, :],
                            start=True, stop=True,
                        )
                        psA = pspool.tile([128, 512], f32, tag="A", bufs=2)
                        for k in range(4):
                            nc.tensor.matmul(
                                psA, lin(bt, k), wiHT[nt][:, k::4, :],
                                start=(k == 0), stop=(k == 3),
                            )
                        psB = pspool.tile([128, 512], f32, tag="B", bufs=2)
                        for k in range(4):
                            nc.tensor.matmul(
                                psB, lmh(bt, k), wHHT[nt][:, k::4, :],
                                start=(k == 0), stop=(k == 3),
                            )

                        mzims = epool.tile([128, 1536], f16, tag="mzims", bufs=2)
                        nc.scalar.activation(mzims, psMM3, ACT.Copy)
                        t1 = epool.tile([128, 512], f16, tag="t1")
                        nc.vector.tensor_mul(t1, psA, mzims[:, 0:512])
                        t2 = epool.tile([128, 512], f16, tag="t2")
                        nc.vector.tensor_mul(t2, psB, mzims[:, 512:1024])
                        t12 = epool.tile([128, 512], f16, tag="t12")
                        if nt in (1, 0):
                            nc.gpsimd.tensor_add(t12, t1, t2)
                        else:
                            nc.vector.tensor_add(t12, t1, t2)
                        pre_s = epool.tile([128, 512], f16, tag="pre")
                        nc.vector.tensor_add(pre_s, t12, mzims[:, 1024:1536])

                        if nt == 1:  # forget gate -> u1 = sig(f) * c
                            gf = epool.tile([128, 512], f16, tag="gout")
                            nc.scalar.activation(gf, pre_s, ACT.Sigmoid)
                            nc.gpsimd.tensor_mul(
                                u1_all[:, bt, :], gf, mc_all[:, bt, :]
                            )
                        elif nt == 0:  # input gate, keep
                            nc.scalar.activation(
                                g0_all[:, bt, :], pre_s, ACT.Sigmoid
                            )
                        elif nt == 2:  # g gate -> c_new, tanh(c_new)
                            tg = epool.tile([128, 512], f16, tag="gout")
                            nc.scalar.activation(tg, pre_s, ACT.Tanh)
                            u2 = epool.tile([128, 512], f16, tag="u2")
                            nc.vector.tensor_mul(u2, g0_all[:, bt, :], tg)
                            c_new = out_c[:, bt, :]
                            nc.vector.tensor_add(c_new, u2, u1_all[:, bt, :])
                            nc.scalar.activation(
                                tc_all[:, bt, :], c_new, ACT.Tanh
                            )
                        else:  # output gate -> h_new
                            go = epool.tile([128, 512], f16, tag="gout")
                            nc.scalar.activation(go, pre_s, ACT.Sigmoid)
                            nc.gpsimd.tensor_mul(
                                out_h[:, bt, :], go, tc_all[:, bt, :]
                            )
                pspool_cm.__exit__(None, None, None)

                # ============ P4: batched cast stores =====================
                nc.gpsimd.dma_start(
                    mcn_d.rearrange("(c p) f -> p c f", p=128), out_c
                )
                nc.gpsimd.dma_start(
                    mhn_d.rearrange("(c p) f -> p c f", p=128), out_h
                )
                nc.gpsimd.dma_start(
                    zcn_d.rearrange("(c p) f -> p c f", p=128), out_zc
                )
                nc.gpsimd.dma_start(
                    zhn_d.rearrange("(c p) f -> p c f", p=128), out_zh
                )

    nc.finalize()
    return nc


def _get_nc(reps=1, mode="full"):
    key = f"nc{reps}{mode}"
    if key not in _CACHE:
        _CACHE[key] = _build(reps, mode)
    return _CACHE[key]


def kernel(**inputs):
    from concourse import bass_utils

    nc = _get_nc()

    arr = {
        k: np.ascontiguousarray(np.asarray(v, dtype=np.float32))
        for k, v in inputs.items()
    }
    arr["bias_i"] = arr["bias_i"].reshape(E_, 1)
    arr["bias_H"] = arr["bias_H"].reshape(E_, 1)
    arr["bias"] = arr["bias"].reshape(1, G4H)
    arr["bias_hyper"] = arr["bias_hyper"].reshape(1, G4HH)

    in_maps = []
    for c in range(N_CORES):
        m = {}
        for k, v in arr.items():
            m[k] = v[c * B_ : (c + 1) * B_] if k in _BATCH_KEYS else v
        in_maps.append(m)

    res = bass_utils.run_bass_kernel_spmd(
        nc, in_maps, core_ids=list(range(N_CORES)), **_CACHE.get("run_kwargs", {})
    )
    _CACHE["last_results"] = res
    outs = res.results
    main_h_new = np.concatenate([outs[c]["main_h_new"] for c in range(N_CORES)], 0)
    main_c_new = np.concatenate([outs[c]["main_c_new"] for c in range(N_CORES)], 0)
    meta_h_new = np.concatenate([outs[c]["meta_h_new"] for c in range(N_CORES)], 0)
    meta_c_new = np.concatenate([outs[c]["meta_c_new"] for c in range(N_CORES)], 0)
    return (main_h_new, main_c_new, meta_h_new, meta_c_new)


# revision 21
# speedup vs baseline: 18864.2897x; 18864.2897x over previous
"""MetaLSTMCell Trainium2 kernel.

Data-parallel over 8 NeuronCores: batch 8192 -> 1024 rows/core, weights
replicated. Per core:
  - fp32 -> fp16 casts happen inside SWDGE DMA loads; fp16 -> fp32 on the
    output stores.
  - Feature-major copies of input/main_h/meta_h and all weights produced by
    ~15 batched DMA XBAR transposes (interleaved chunk layouts + strided
    APs). The three dz hypernet weights + main bias are staged as stacked
    columns so ONE transpose yields the [97, 2048] stacked operand whose
    partition rows line up with the stacked z vector for row-packed K=32
    matmuls.
  - Matmuls fp16, fp32 PSUM accumulation, N=512 moving tiles.
  - Gate assembly pre = Mzi*A + MzH*B + C(+bias) spread across DVE/ACT/POOL.
"""

import sys

import numpy as np

if "/opt/trn_rl_repo" not in sys.path:
    sys.path.insert(0, "/opt/trn_rl_repo")

I_, H_, HH_, E_ = 512, 512, 128, 32
G4H, G4HH = 4 * H_, 4 * HH_  # 2048, 512
N_CORES = 8
B_FULL = 8192
B_ = B_FULL // N_CORES  # 1024 rows per core
NBT = B_ // 128  # 8 batch tiles per core

_BATCH_KEYS = ("input", "main_h", "main_c", "meta_h", "meta_c")

_CACHE = {}


def _build(reps=1, mode="full"):
    import concourse.bacc as bacc
    import concourse.mybir as mybir
    import concourse.tile as tile

    f16 = mybir.dt.float16
    f32 = mybir.dt.float32
    ACT = mybir.ActivationFunctionType

    nc = bacc.Bacc(
        "TRN2", target_bir_lowering=False, debug=False, enable_asserts=False
    )

    def gload(out, in_):
        if mode != "comp":
            nc.gpsimd.dma_start(out, in_)
        else:
            nc.gpsimd.memset(out, 0.25)

    # ---- DRAM I/O ------------------------------------------------------
    x_d = nc.dram_tensor("input", [B_, I_], f32, kind="ExternalInput").ap()
    mh_d = nc.dram_tensor("main_h", [B_, H_], f32, kind="ExternalInput").ap()
    mc_d = nc.dram_tensor("main_c", [B_, H_], f32, kind="ExternalInput").ap()
    zh_d = nc.dram_tensor("meta_h", [B_, HH_], f32, kind="ExternalInput").ap()
    zc_d = nc.dram_tensor("meta_c", [B_, HH_], f32, kind="ExternalInput").ap()

    wiH_d = nc.dram_tensor("weight_iH", [G4H, I_], f32, kind="ExternalInput").ap()
    wHH_d = nc.dram_tensor("weight_HH", [G4H, H_], f32, kind="ExternalInput").ap()
    wih_d = nc.dram_tensor("weight_ih", [G4HH, I_ + H_], f32, kind="ExternalInput").ap()
    whh_d = nc.dram_tensor("weight_hh", [G4HH, HH_], f32, kind="ExternalInput").ap()
    whzi_d = nc.dram_tensor("weight_hzi", [E_, HH_], f32, kind="ExternalInput").ap()
    whzH_d = nc.dram_tensor("weight_hzH", [E_, HH_], f32, kind="ExternalInput").ap()
    whzb_d = nc.dram_tensor("weight_hzb", [E_, HH_], f32, kind="ExternalInput").ap()
    wdzi_d = nc.dram_tensor("weight_dziH", [G4H, E_], f32, kind="ExternalInput").ap()
    wdzH_d = nc.dram_tensor("weight_dzHH", [G4H, E_], f32, kind="ExternalInput").ap()
    wbz_d = nc.dram_tensor("weight_bzH", [G4H, E_], f32, kind="ExternalInput").ap()
    bias_i_d = nc.dram_tensor("bias_i", [E_, 1], f32, kind="ExternalInput").ap()
    bias_H_d = nc.dram_tensor("bias_H", [E_, 1], f32, kind="ExternalInput").ap()
    bias_d = nc.dram_tensor("bias", [1, G4H], f32, kind="ExternalInput").ap()
    bias_hy_d = nc.dram_tensor("bias_hyper", [1, G4HH], f32, kind="ExternalInput").ap()

    mhn_d = nc.dram_tensor("main_h_new", [B_, H_], f32, kind="ExternalOutput").ap()
    mcn_d = nc.dram_tensor("main_c_new", [B_, H_], f32, kind="ExternalOutput").ap()
    zhn_d = nc.dram_tensor("meta_h_new", [B_, HH_], f32, kind="ExternalOutput").ap()
    zcn_d = nc.dram_tensor("meta_c_new", [B_, HH_], f32, kind="ExternalOutput").ap()

    with tile.TileContext(nc) as tc:
        with (
            tc.tile_pool(name="wpool", bufs=1) as wpool,
            tc.tile_pool(name="apool", bufs=1) as apool,
            tc.tile_pool(name="epool", bufs=3) as epool,
            tc.tile_pool(name="gpool", bufs=2) as gpool,
            tc.tile_pool(name="opool", bufs=1) as opool,
        ):
            for rep_ in range(reps):
                # ============ P0a: activations needed for matmuls =========
                zh_all = apool.tile([128, NBT, HH_], f16)
                gload(zh_all, zh_d.rearrange("(c p) f -> p c f", p=128))
                in_all = apool.tile([128, NBT, I_], f16)
                gload(in_all, x_d.rearrange("(c p) f -> p c f", p=128))
                mh_all = apool.tile([128, NBT, H_], f16)
                gload(mh_all, mh_d.rearrange("(c p) f -> p c f", p=128))

                # transposes of those (one DMA each)
                meta_hT = wpool.tile([128, B_], f16)
                nc.sync.dma_start_transpose(
                    meta_hT.rearrange("p (c q) -> p c q", q=128), zh_all[:, :, :]
                )
                inT_all = wpool.tile([128, 4 * NBT, 128], f16)
                nc.sync.dma_start_transpose(inT_all, in_all[:, :, :])
                mhT_all = wpool.tile([128, 4 * NBT, 128], f16)
                nc.sync.dma_start_transpose(mhT_all, mh_all[:, :, :])

                # ============ P0b: z/meta path weights ====================
                # small z weights [32, 128] -> [128, 32]
                shz = []
                for j, wd in enumerate((whzi_d, whzH_d, whzb_d)):
                    s = wpool.tile([E_, HH_], f16, tag=f"shz{j}", name=f"shz{j}")
                    gload(s, wd)
                    shz.append(s)
                whzT = []
                for j in range(3):
                    t = wpool.tile([128, E_], f16, tag=f"whzT{j}", name=f"whzT{j}")
                    nc.sync.dma_start_transpose(t, shz[j])
                    whzT.append(t)

                bias_i_sb = wpool.tile([E_, 1], f32)
                if mode != "comp":
                    nc.sync.dma_start(bias_i_sb, bias_i_d)
                else:
                    nc.vector.memset(bias_i_sb, 0.1)
                bias_H_sb = wpool.tile([E_, 1], f32)
                if mode != "comp":
                    nc.sync.dma_start(bias_H_sb, bias_H_d)
                else:
                    nc.vector.memset(bias_H_sb, 0.1)
                bias_hy_sb = wpool.tile([1, G4HH], f16)
                gload(bias_hy_sb, bias_hy_d)
                ones1 = wpool.tile([1, 128], f16)
                nc.vector.memset(ones1, 1.0)

                # meta weights
                sih = wpool.tile(
                    [128, 4, I_ + H_], f16, tag="wstage", name="sih", bufs=2
                )
                gload(sih, wih_d.rearrange("(c p) f -> p c f", p=128))
                wihT = wpool.tile([128, 32, 128], f16)
                nc.sync.dma_start_transpose(wihT, sih[:, :, :])
                shh = wpool.tile([128, 4, HH_], f16, tag="wstage", name="shh", bufs=2)
                gload(shh, whh_d.rearrange("(c p) f -> p c f", p=128))
                whhT = wpool.tile([128, 4, 128], f16)
                nc.sync.dma_start_transpose(whhT, shh[:, :, :])

                # dz weights + main bias staged as stacked columns:
                # sdz3[:, c, 0:32]=Wdzi | 32:64=WdzH | 64:96=Wbz | 96=bias
                sdz3_32 = wpool.tile([128, 16, 128], f32)
                nc.vector.memset(sdz3_32, 0.0)
                if mode != "comp":
                    nc.sync.dma_start(
                        sdz3_32[:, :, 0:E_],
                        wdzi_d.rearrange("(c p) e -> p c e", p=128),
                    )
                    nc.sync.dma_start(
                        sdz3_32[:, :, E_ : 2 * E_],
                        wdzH_d.rearrange("(c p) e -> p c e", p=128),
                    )
                    nc.sync.dma_start(
                        sdz3_32[:, :, 2 * E_ : 3 * E_],
                        wbz_d.rearrange("(c p) e -> p c e", p=128),
                    )
                    nc.sync.dma_start(
                        sdz3_32[:, :, 3 * E_ : 3 * E_ + 1],
                        bias_d.rearrange("x (c p) -> p c x", p=128),
                    )
                sdz3 = wpool.tile([128, 16, 128], f16)
                nc.vector.tensor_copy(sdz3, sdz3_32)
                # one transpose -> [128, 16, 128]: rows 0:32 Wdzi^T, 32:64
                # WdzH^T, 64:96 Wbz^T, row 96 bias; cols (c, q): g = 128c+q
                wdzT = wpool.tile([128, 16, 128], f16)
                nc.sync.dma_start_transpose(wdzT, sdz3[:, :, :])

                # ============ P0c: big main weights =======================
                wiHT = [
                    wpool.tile([128, 16, 128], f16, tag=f"wiHT{j}", name=f"wiHT{j}")
                    for j in range(4)
                ]
                wHHT = [
                    wpool.tile([128, 16, 128], f16, tag=f"wHHT{j}", name=f"wHHT{j}")
                    for j in range(4)
                ]
                for h in range(2):
                    s = wpool.tile(
                        [128, 8, I_], f16, tag="wstage", name=f"siH{h}", bufs=2
                    )
                    gload(
                        s,
                        wiH_d[1024 * h : 1024 * (h + 1), :].rearrange(
                            "(c p) f -> p c f", p=128
                        ),
                    )
                    for i in range(2):
                        nc.sync.dma_start_transpose(
                            wiHT[2 * h + i], s[:, 4 * i : 4 * i + 4, :]
                        )
                    s = wpool.tile(
                        [128, 8, H_], f16, tag="wstage", name=f"sHH{h}", bufs=2
                    )
                    gload(
                        s,
                        wHH_d[1024 * h : 1024 * (h + 1), :].rearrange(
                            "(c p) f -> p c f", p=128
                        ),
                    )
                    for i in range(2):
                        nc.sync.dma_start_transpose(
                            wHHT[2 * h + i], s[:, 4 * i : 4 * i + 4, :]
                        )

                # ============ P0d: elementwise-only data ==================
                zc_all = apool.tile([128, NBT, HH_], f16)
                gload(zc_all, zc_d.rearrange("(c p) f -> p c f", p=128))
                mc_all = apool.tile([128, NBT, H_], f16)
                gload(mc_all, mc_d.rearrange("(c p) f -> p c f", p=128))

                # fp16 output accumulation tiles (one cast store at the end)
                out_c = opool.tile([128, NBT, H_], f16)
                out_h = opool.tile([128, NBT, H_], f16)
                out_zc = opool.tile([128, NBT, HH_], f16)
                out_zh = opool.tile([128, NBT, HH_], f16)

                # ============ P2: hypernet z path =========================
                with tc.tile_pool(name="zpool", bufs=1, space="PSUM") as zpool:
                    z_ps = []
                    for j in range(3):
                        zp = zpool.tile([E_, B_], f32, tag=f"z{j}", name=f"z{j}")
                        for h in range(B_ // 512):
                            nc.tensor.matmul(
                                zp[:, 512 * h : 512 * (h + 1)],
                                whzT[j],
                                meta_hT[:, 512 * h : 512 * (h + 1)],
                                start=True,
                                stop=True,
                            )
                        z_ps.append(zp)

                    # zstack rows: 0:32 zi | 32:64 zH | 64:96 zb | 96 ones
                    zstack = wpool.tile([97, B_], f16)
                    nc.vector.tensor_scalar_add(zstack[0:E_, :], z_ps[0], bias_i_sb)
                    nc.vector.tensor_scalar_add(
                        zstack[E_ : 2 * E_, :], z_ps[1], bias_H_sb
                    )
                    nc.scalar.activation(zstack[2 * E_ : 3 * E_, :], z_ps[2], ACT.Copy)
                    nc.vector.memset(zstack[3 * E_ : 3 * E_ + 1, :], 1.0)

                # ============ P3: gate-major compute ======================
                pspool_cm = tc.tile_pool(name="pspool", bufs=1, space="PSUM")
                pspool = pspool_cm.__enter__()

                def lin(bt, k):
                    return inT_all[:, 4 * bt + k, :]

                def lmh(bt, k):
                    return mhT_all[:, 4 * bt + k, :]

                # ---- meta (hyper) LSTM: early PE work while weights load --
                for bt in range(NBT):
                    btsl = slice(bt * 128, (bt + 1) * 128)
                    ps_meta = pspool.tile([128, G4HH], f32, tag="meta", bufs=1)
                    for k in range(4):
                        nc.tensor.matmul(
                            ps_meta, lin(bt, k), wihT[:, k::8, :],
                            start=(k == 0), stop=False,
                        )
                    for k in range(4):
                        nc.tensor.matmul(
                            ps_meta, lmh(bt, k), wihT[:, 4 + k :: 8, :],
                            start=False, stop=False,
                        )
                    nc.tensor.matmul(
                        ps_meta, meta_hT[:, btsl], whhT, start=False, stop=False
                    )
                    nc.tensor.matmul(
                        ps_meta, ones1, bias_hy_sb, start=False, stop=True
                    )

                    smi = epool.tile([128, HH_], f16, tag="smi")
                    nc.scalar.activation(smi, ps_meta[:, 0:128], ACT.Sigmoid)
                    smf = epool.tile([128, HH_], f16, tag="smf")
                    nc.scalar.activation(smf, ps_meta[:, 128:256], ACT.Sigmoid)
                    tmg = epool.tile([128, HH_], f16, tag="tmg")
                    nc.scalar.activation(tmg, ps_meta[:, 256:384], ACT.Tanh)
                    smo = epool.tile([128, HH_], f16, tag="smo")
                    nc.scalar.activation(smo, ps_meta[:, 384:512], ACT.Sigmoid)

                    v1 = epool.tile([128, HH_], f16, tag="v1")
                    nc.gpsimd.tensor_mul(v1, smf, zc_all[:, bt, :])
                    v2 = epool.tile([128, HH_], f16, tag="v2")
                    nc.gpsimd.tensor_mul(v2, smi, tmg)
                    zc_new = out_zc[:, bt, :]
                    nc.vector.tensor_add(zc_new, v1, v2)
                    tmc = epool.tile([128, HH_], f16, tag="tmc")
                    nc.scalar.activation(tmc, zc_new, ACT.Tanh)
                    nc.gpsimd.tensor_mul(out_zh[:, bt, :], smo, tmc)

                # ---- main LSTM gates, nt-major, combine fused in ----
                # order: f(1) -> i(0) -> g(2) -> o(3)
                g0_all = opool.tile([128, NBT, 512], f16)
                u1_all = opool.tile([128, NBT, 512], f16)
                tc_all = opool.tile([128, NBT, 512], f16)
                for nt in [1, 0, 2, 3]:
                    for bt in range(NBT):
                        btsl = slice(bt * 128, (bt + 1) * 128)
                        psMM3 = pspool.tile([128, 1536], f32, tag="MM3", bufs=1)
                        nc.tensor.matmul(
                            psMM3[:, 0:512],
                            zstack[0:E_, btsl],
                            wdzT[0:E_, 4 * nt : 4 * nt + 4, :],
                            start=True, stop=True,
                        )
                        nc.tensor.matmul(
                            psMM3[:, 512:1024],
                            zstack[E_ : 2 * E_, btsl],
                            wdzT[E_ : 2 * E_, 4 * nt : 4 * nt + 4, :],
                            start=True, stop=True,
                        )
                        nc.tensor.matmul(
                            psMM3[:, 1024:1536],
                            zstack[2 * E_ : 3 * E_ + 1, btsl],
                            wdzT[2 * E_ : 3 * E_ + 1, 4 * nt : 4 * nt + 4, :],
                            start=True, stop=True,
                        )
                        psA = pspool.tile([128, 512], f32, tag="A", bufs=2)
                        for k in range(4):
                            nc.tensor.matmul(
                                psA, lin(bt, k), wiHT[nt][:, k::4, :],
                                start=(k == 0), stop=(k == 3),
                            )
                        psB = pspool.tile([128, 512], f32, tag="B", bufs=2)
                        for k in range(4):
                            nc.tensor.matmul(
                                psB, lmh(bt, k), wHHT[nt][:, k::4, :],
                                start=(k == 0), stop=(k == 3),
                            )

                        mzims = epool.tile([128, 1536], f16, tag="mzims", bufs=2)
                        nc.scalar.activation(mzims, psMM3, ACT.Copy)
                        t1 = epool.tile([128, 512], f16, tag="t1")
                        nc.vector.tensor_mul(t1, psA, mzims[:, 0:512])
                        t2 = epool.tile([128, 512], f16, tag="t2")
                        nc.vector.tensor_mul(t2, psB, mzims[:, 512:1024])
                        t12 = epool.tile([128, 512], f16, tag="t12")
                        if nt in (1, 0):
                            nc.gpsimd.tensor_add(t12, t1, t2)
                        else:
                            nc.vector.tensor_add(t12, t1, t2)
                        pre_s = epool.tile([128, 512], f16, tag="pre")
                        nc.vector.tensor_add(pre_s, t12, mzims[:, 1024:1536])

                        if nt == 1:  # forget gate -> u1 = sig(f) * c
                            gf = epool.tile([128, 512], f16, tag="gout")
                            nc.scalar.activation(gf, pre_s, ACT.Sigmoid)
                            nc.gpsimd.tensor_mul(
                                u1_all[:, bt, :], gf, mc_all[:, bt, :]
                            )
                        elif nt == 0:  # input gate, keep
                            nc.scalar.activation(
                                g0_all[:, bt, :], pre_s, ACT.Sigmoid
                            )
                        elif nt == 2:  # g gate -> c_new, tanh(c_new)
                            tg = epool.tile([128, 512], f16, tag="gout")
                            nc.scalar.activation(tg, pre_s, ACT.Tanh)
                            u2 = epool.tile([128, 512], f16, tag="u2")
                            nc.vector.tensor_mul(u2, g0_all[:, bt, :], tg)
                            c_new = out_c[:, bt, :]
                            nc.vector.tensor_add(c_new, u2, u1_all[:, bt, :])
                            nc.scalar.activation(
                                tc_all[:, bt, :], c_new, ACT.Tanh
                            )
                        else:  # output gate -> h_new
                            go = epool.tile([128, 512], f16, tag="gout")
                            nc.scalar.activation(go, pre_s, ACT.Sigmoid)
                            nc.gpsimd.tensor_mul(
                                out_h[:, bt, :], go, tc_all[:, bt, :]
                            )
                pspool_cm.__exit__(None, None, None)

                # ============ P4: batched cast stores =====================
                nc.gpsimd.dma_start(
                    mcn_d.rearrange("(c p) f -> p c f", p=128), out_c
                )
                nc.gpsimd.dma_start(
                    mhn_d.rearrange("(c p) f -> p c f", p=128), out_h
                )
                nc.gpsimd.dma_start(
                    zcn_d.rearrange("(c p) f -> p c f", p=128), out_zc
                )
                nc.gpsimd.dma_start(
                    zhn_d.rearrange("(c p) f -> p c f", p=128), out_zh
                )

    nc.finalize()
    return nc


def _get_nc(reps=1, mode="full"):
    key = f"nc{reps}{mode}"
    if key not in _CACHE:
        _CACHE[key] = _build(reps, mode)
    return _CACHE[key]


def kernel(**inputs):
    from concourse import bass_utils

    nc = _get_nc()

    arr = {
        k: np.ascontiguousarray(np.asarray(v, dtype=np.float32))
        for k, v in inputs.items()
    }
    arr["bias_i"] = arr["bias_i"].reshape(E_, 1)
    arr["bias_H"] = arr["bias_H"].reshape(E_, 1)
    arr["bias"] = arr["bias"].reshape(1, G4H)
    arr["bias_hyper"] = arr["bias_hyper"].reshape(1, G4HH)

    in_maps = []
    for c in range(N_CORES):
        m = {}
        for k, v in arr.items():
            m[k] = v[c * B_ : (c + 1) * B_] if k in _BATCH_KEYS else v
        in_maps.append(m)

    res = bass_utils.run_bass_kernel_spmd(
        nc, in_maps, core_ids=list(range(N_CORES)), **_CACHE.get("run_kwargs", {})
    )
    _CACHE["last_results"] = res
    outs = res.results
    main_h_new = np.concatenate([outs[c]["main_h_new"] for c in range(N_CORES)], 0)
    main_c_new = np.concatenate([outs[c]["main_c_new"] for c in range(N_CORES)], 0)
    meta_h_new = np.concatenate([outs[c]["meta_h_new"] for c in range(N_CORES)], 0)
    meta_c_new = np.concatenate([outs[c]["meta_c_new"] for c in range(N_CORES)], 0)
    return (main_h_new, main_c_new, meta_h_new, meta_c_new)


# revision 22
# speedup vs baseline: 29129.3242x; 1.5442x over previous
"""MetaLSTMCell Trainium2 kernel.

Data-parallel over 8 NeuronCores: batch 8192 -> 1024 rows/core, weights
replicated. Per core:
  - fp32 -> fp16 casts happen inside SWDGE DMA loads; fp16 -> fp32 on the
    output stores.
  - Feature-major copies of input/main_h/meta_h and all weights produced by
    ~15 batched DMA XBAR transposes (interleaved chunk layouts + strided
    APs). The three dz hypernet weights + main bias are staged as stacked
    columns so ONE transpose yields the [97, 2048] stacked operand whose
    partition rows line up with the stacked z vector for row-packed K=32
    matmuls.
  - Matmuls fp16, fp32 PSUM accumulation, N=512 moving tiles.
  - Gate assembly pre = Mzi*A + MzH*B + C(+bias) spread across DVE/ACT/POOL.
"""

import sys

import numpy as np

if "/opt/trn_rl_repo" not in sys.path:
    sys.path.insert(0, "/opt/trn_rl_repo")

I_, H_, HH_, E_ = 512, 512, 128, 32
G4H, G4HH = 4 * H_, 4 * HH_  # 2048, 512
N_CORES = 8
B_FULL = 8192
B_ = B_FULL // N_CORES  # 1024 rows per core
NBT = B_ // 128  # 8 batch tiles per core

_BATCH_KEYS = ("input", "main_h", "main_c", "meta_h", "meta_c")

_CACHE = {}


def _build(reps=1, mode="full"):
    import concourse.bacc as bacc
    import concourse.mybir as mybir
    import concourse.tile as tile

    f16 = mybir.dt.float16
    f32 = mybir.dt.float32
    ACT = mybir.ActivationFunctionType

    nc = bacc.Bacc(
        "TRN2", target_bir_lowering=False, debug=False, enable_asserts=False
    )

    def gload(out, in_):
        if mode != "comp":
            nc.gpsimd.dma_start(out, in_)
        else:
            nc.gpsimd.memset(out, 0.25)

    # ---- DRAM I/O ------------------------------------------------------
    x_d = nc.dram_tensor("input", [B_, I_], f32, kind="ExternalInput").ap()
    mh_d = nc.dram_tensor("main_h", [B_, H_], f32, kind="ExternalInput").ap()
    mc_d = nc.dram_tensor("main_c", [B_, H_], f32, kind="ExternalInput").ap()
    zh_d = nc.dram_tensor("meta_h", [B_, HH_], f32, kind="ExternalInput").ap()
    zc_d = nc.dram_tensor("meta_c", [B_, HH_], f32, kind="ExternalInput").ap()

    wiH_d = nc.dram_tensor("weight_iH", [G4H, I_], f32, kind="ExternalInput").ap()
    wHH_d = nc.dram_tensor("weight_HH", [G4H, H_], f32, kind="ExternalInput").ap()
    wih_d = nc.dram_tensor("weight_ih", [G4HH, I_ + H_], f32, kind="ExternalInput").ap()
    whh_d = nc.dram_tensor("weight_hh", [G4HH, HH_], f32, kind="ExternalInput").ap()
    whzi_d = nc.dram_tensor("weight_hzi", [E_, HH_], f32, kind="ExternalInput").ap()
    whzH_d = nc.dram_tensor("weight_hzH", [E_, HH_], f32, kind="ExternalInput").ap()
    whzb_d = nc.dram_tensor("weight_hzb", [E_, HH_], f32, kind="ExternalInput").ap()
    wdzi_d = nc.dram_tensor("weight_dziH", [G4H, E_], f32, kind="ExternalInput").ap()
    wdzH_d = nc.dram_tensor("weight_dzHH", [G4H, E_], f32, kind="ExternalInput").ap()
    wbz_d = nc.dram_tensor("weight_bzH", [G4H, E_], f32, kind="ExternalInput").ap()
    bias_i_d = nc.dram_tensor("bias_i", [E_, 1], f32, kind="ExternalInput").ap()
    bias_H_d = nc.dram_tensor("bias_H", [E_, 1], f32, kind="ExternalInput").ap()
    bias_d = nc.dram_tensor("bias", [1, G4H], f32, kind="ExternalInput").ap()
    bias_hy_d = nc.dram_tensor("bias_hyper", [1, G4HH], f32, kind="ExternalInput").ap()

    mhn_d = nc.dram_tensor("main_h_new", [B_, H_], f32, kind="ExternalOutput").ap()
    mcn_d = nc.dram_tensor("main_c_new", [B_, H_], f32, kind="ExternalOutput").ap()
    zhn_d = nc.dram_tensor("meta_h_new", [B_, HH_], f32, kind="ExternalOutput").ap()
    zcn_d = nc.dram_tensor("meta_c_new", [B_, HH_], f32, kind="ExternalOutput").ap()

    with tile.TileContext(nc) as tc:
        with (
            tc.tile_pool(name="wpool", bufs=1) as wpool,
            tc.tile_pool(name="apool", bufs=1) as apool,
            tc.tile_pool(name="epool", bufs=4) as epool,
            tc.tile_pool(name="gpool", bufs=2) as gpool,
            tc.tile_pool(name="opool", bufs=1) as opool,
        ):
            for rep_ in range(reps):
                # ============ P0a: activations needed for matmuls =========
                zh_all = apool.tile([128, NBT, HH_], f16)
                gload(zh_all, zh_d.rearrange("(c p) f -> p c f", p=128))
                in_all = apool.tile([128, NBT, I_], f16)
                gload(in_all, x_d.rearrange("(c p) f -> p c f", p=128))
                mh_all = apool.tile([128, NBT, H_], f16)
                gload(mh_all, mh_d.rearrange("(c p) f -> p c f", p=128))

                # transposes of those (one DMA each)
                meta_hT = wpool.tile([128, B_], f16)
                nc.sync.dma_start_transpose(
                    meta_hT.rearrange("p (c q) -> p c q", q=128), zh_all[:, :, :]
                )
                inT_all = wpool.tile([128, 4 * NBT, 128], f16)
                nc.sync.dma_start_transpose(inT_all, in_all[:, :, :])
                mhT_all = wpool.tile([128, 4 * NBT, 128], f16)
                nc.sync.dma_start_transpose(mhT_all, mh_all[:, :, :])

                # ============ P0b: z/meta path weights ====================
                # small z weights [32, 128] -> [128, 32]
                shz = []
                for j, wd in enumerate((whzi_d, whzH_d, whzb_d)):
                    s = wpool.tile([E_, HH_], f16, tag=f"shz{j}", name=f"shz{j}")
                    gload(s, wd)
                    shz.append(s)
                whzT = []
                for j in range(3):
                    t = wpool.tile([128, E_], f16, tag=f"whzT{j}", name=f"whzT{j}")
                    nc.sync.dma_start_transpose(t, shz[j])
                    whzT.append(t)

                bias_i_sb = wpool.tile([E_, 1], f32)
                if mode != "comp":
                    nc.sync.dma_start(bias_i_sb, bias_i_d)
                else:
                    nc.vector.memset(bias_i_sb, 0.1)
                bias_H_sb = wpool.tile([E_, 1], f32)
                if mode != "comp":
                    nc.sync.dma_start(bias_H_sb, bias_H_d)
                else:
                    nc.vector.memset(bias_H_sb, 0.1)
                bias_hy_sb = wpool.tile([1, G4HH], f16)
                gload(bias_hy_sb, bias_hy_d)
                ones1 = wpool.tile([1, 128], f16)
                nc.vector.memset(ones1, 1.0)

                # meta weights
                sih = wpool.tile(
                    [128, 4, I_ + H_], f16, tag="wstage", name="sih", bufs=2
                )
                gload(sih, wih_d.rearrange("(c p) f -> p c f", p=128))
                wihT = wpool.tile([128, 32, 128], f16)
                nc.sync.dma_start_transpose(wihT, sih[:, :, :])
                shh = wpool.tile([128, 4, HH_], f16, tag="wstage", name="shh", bufs=2)
                gload(shh, whh_d.rearrange("(c p) f -> p c f", p=128))
                whhT = wpool.tile([128, 4, 128], f16)
                nc.sync.dma_start_transpose(whhT, shh[:, :, :])

                # dz weights + main bias staged as stacked columns:
                # sdz3[:, c, 0:32]=Wdzi | 32:64=WdzH | 64:96=Wbz | 96=bias
                sdz3_32 = wpool.tile([128, 16, 128], f32)
                nc.vector.memset(sdz3_32, 0.0)
                if mode != "comp":
                    nc.sync.dma_start(
                        sdz3_32[:, :, 0:E_],
                        wdzi_d.rearrange("(c p) e -> p c e", p=128),
                    )
                    nc.sync.dma_start(
                        sdz3_32[:, :, E_ : 2 * E_],
                        wdzH_d.rearrange("(c p) e -> p c e", p=128),
                    )
                    nc.sync.dma_start(
                        sdz3_32[:, :, 2 * E_ : 3 * E_],
                        wbz_d.rearrange("(c p) e -> p c e", p=128),
                    )
                    nc.sync.dma_start(
                        sdz3_32[:, :, 3 * E_ : 3 * E_ + 1],
                        bias_d.rearrange("x (c p) -> p c x", p=128),
                    )
                sdz3 = wpool.tile([128, 16, 128], f16)
                nc.vector.tensor_copy(sdz3, sdz3_32)
                # one transpose -> [128, 16, 128]: rows 0:32 Wdzi^T, 32:64
                # WdzH^T, 64:96 Wbz^T, row 96 bias; cols (c, q): g = 128c+q
                wdzT = wpool.tile([128, 16, 128], f16)
                nc.sync.dma_start_transpose(wdzT, sdz3[:, :, :])

                # ============ P0c: big main weights =======================
                wiHT = [
                    wpool.tile([128, 16, 128], f16, tag=f"wiHT{j}", name=f"wiHT{j}")
                    for j in range(4)
                ]
                wHHT = [
                    wpool.tile([128, 16, 128], f16, tag=f"wHHT{j}", name=f"wHHT{j}")
                    for j in range(4)
                ]
                for h in range(2):
                    s = wpool.tile(
                        [128, 8, I_], f16, tag="wstage", name=f"siH{h}", bufs=2
                    )
                    gload(
                        s,
                        wiH_d[1024 * h : 1024 * (h + 1), :].rearrange(
                            "(c p) f -> p c f", p=128
                        ),
                    )
                    for i in range(2):
                        nc.sync.dma_start_transpose(
                            wiHT[2 * h + i], s[:, 4 * i : 4 * i + 4, :]
                        )
                    s = wpool.tile(
                        [128, 8, H_], f16, tag="wstage", name=f"sHH{h}", bufs=2
                    )
                    gload(
                        s,
                        wHH_d[1024 * h : 1024 * (h + 1), :].rearrange(
                            "(c p) f -> p c f", p=128
                        ),
                    )
                    for i in range(2):
                        nc.sync.dma_start_transpose(
                            wHHT[2 * h + i], s[:, 4 * i : 4 * i + 4, :]
                        )

                # ============ P0d: elementwise-only data ==================
                zc_all = apool.tile([128, NBT, HH_], f16)
                gload(zc_all, zc_d.rearrange("(c p) f -> p c f", p=128))
                mc_all = apool.tile([128, NBT, H_], f16)
                gload(mc_all, mc_d.rearrange("(c p) f -> p c f", p=128))

                # fp16 output accumulation tiles (one cast store at the end)
                out_c = opool.tile([128, NBT, H_], f16)
                out_h = opool.tile([128, NBT, H_], f16)
                out_zc = opool.tile([128, NBT, HH_], f16)
                out_zh = opool.tile([128, NBT, HH_], f16)

                # ============ P2: hypernet z path =========================
                with tc.tile_pool(name="zpool", bufs=1, space="PSUM") as zpool:
                    z_ps = []
                    for j in range(3):
                        zp = zpool.tile([E_, B_], f32, tag=f"z{j}", name=f"z{j}")
                        for h in range(B_ // 512):
                            nc.tensor.matmul(
                                zp[:, 512 * h : 512 * (h + 1)],
                                whzT[j],
                                meta_hT[:, 512 * h : 512 * (h + 1)],
                                start=True,
                                stop=True,
                            )
                        z_ps.append(zp)

                    # zstack rows: 0:32 zi | 32:64 zH | 64:96 zb | 96 ones
                    zstack = wpool.tile([97, B_], f16)
                    nc.vector.tensor_scalar_add(zstack[0:E_, :], z_ps[0], bias_i_sb)
                    nc.vector.tensor_scalar_add(
                        zstack[E_ : 2 * E_, :], z_ps[1], bias_H_sb
                    )
                    nc.scalar.activation(zstack[2 * E_ : 3 * E_, :], z_ps[2], ACT.Copy)
                    nc.vector.memset(zstack[3 * E_ : 3 * E_ + 1, :], 1.0)

                # ============ P3: gate-major compute ======================
                pspool_cm = tc.tile_pool(name="pspool", bufs=1, space="PSUM")
                pspool = pspool_cm.__enter__()

                def lin(bt, k):
                    return inT_all[:, 4 * bt + k, :]

                def lmh(bt, k):
                    return mhT_all[:, 4 * bt + k, :]

                # ---- meta (hyper) LSTM: early PE work while weights load --
                for bt in range(NBT):
                    btsl = slice(bt * 128, (bt + 1) * 128)
                    ps_meta = pspool.tile([128, G4HH], f32, tag="meta", bufs=1)
                    for k in range(4):
                        nc.tensor.matmul(
                            ps_meta, lin(bt, k), wihT[:, k::8, :],
                            start=(k == 0), stop=False,
                        )
                    for k in range(4):
                        nc.tensor.matmul(
                            ps_meta, lmh(bt, k), wihT[:, 4 + k :: 8, :],
                            start=False, stop=False,
                        )
                    nc.tensor.matmul(
                        ps_meta, meta_hT[:, btsl], whhT, start=False, stop=False
                    )
                    nc.tensor.matmul(
                        ps_meta, ones1, bias_hy_sb, start=False, stop=True
                    )

                    smi = epool.tile([128, HH_], f16, tag="smi")
                    nc.scalar.activation(smi, ps_meta[:, 0:128], ACT.Sigmoid)
                    smf = epool.tile([128, HH_], f16, tag="smf")
                    nc.scalar.activation(smf, ps_meta[:, 128:256], ACT.Sigmoid)
                    tmg = epool.tile([128, HH_], f16, tag="tmg")
                    nc.scalar.activation(tmg, ps_meta[:, 256:384], ACT.Tanh)
                    smo = epool.tile([128, HH_], f16, tag="smo")
                    nc.scalar.activation(smo, ps_meta[:, 384:512], ACT.Sigmoid)

                    v1 = epool.tile([128, HH_], f16, tag="v1")
                    nc.gpsimd.tensor_mul(v1, smf, zc_all[:, bt, :])
                    v2 = epool.tile([128, HH_], f16, tag="v2")
                    nc.gpsimd.tensor_mul(v2, smi, tmg)
                    zc_new = out_zc[:, bt, :]
                    nc.vector.tensor_add(zc_new, v1, v2)
                    tmc = epool.tile([128, HH_], f16, tag="tmc")
                    nc.scalar.activation(tmc, zc_new, ACT.Tanh)
                    nc.gpsimd.tensor_mul(out_zh[:, bt, :], smo, tmc)

                # ---- main LSTM gates, nt-major, combine fused in ----
                # order: f(1) -> i(0) -> g(2) -> o(3)
                g0_all = opool.tile([128, NBT, 512], f16)
                u1_all = opool.tile([128, NBT, 512], f16)
                tc_all = opool.tile([128, NBT, 512], f16)
                for nt in [1, 0, 2, 3]:
                    for bt in range(NBT):
                        btsl = slice(bt * 128, (bt + 1) * 128)
                        psMM3 = pspool.tile([128, 1536], f32, tag="MM3", bufs=1)
                        nc.tensor.matmul(
                            psMM3[:, 0:512],
                            zstack[0:E_, btsl],
                            wdzT[0:E_, 4 * nt : 4 * nt + 4, :],
                            start=True, stop=True,
                        )
                        nc.tensor.matmul(
                            psMM3[:, 512:1024],
                            zstack[E_ : 2 * E_, btsl],
                            wdzT[E_ : 2 * E_, 4 * nt : 4 * nt + 4, :],
                            start=True, stop=True,
                        )
                        nc.tensor.matmul(
                            psMM3[:, 1024:1536],
                            zstack[2 * E_ : 3 * E_ + 1, btsl],
                            wdzT[2 * E_ : 3 * E_ + 1, 4 * nt : 4 * nt + 4, :],
                            start=True, stop=True,
                        )
                        psA = pspool.tile([128, 512], f32, tag="A", bufs=2)
                        for k in range(4):
                            nc.tensor.matmul(
                                psA, lin(bt, k), wiHT[nt][:, k::4, :],
                                start=(k == 0), stop=(k == 3),
                            )
                        psB = pspool.tile([128, 512], f32, tag="B", bufs=2)
                        for k in range(4):
                            nc.tensor.matmul(
                                psB, lmh(bt, k), wHHT[nt][:, k::4, :],
                                start=(k == 0), stop=(k == 3),
                            )

                        mzims = epool.tile([128, 1536], f16, tag="mzims", bufs=3)
                        nc.scalar.activation(mzims, psMM3, ACT.Copy)
                        t1 = epool.tile([128, 512], f16, tag="t1")
                        nc.vector.tensor_mul(t1, psA, mzims[:, 0:512])
                        t2 = epool.tile([128, 512], f16, tag="t2")
                        nc.vector.tensor_mul(t2, psB, mzims[:, 512:1024])
                        t12 = epool.tile([128, 512], f16, tag="t12")
                        if nt in (1, 0):
                            nc.gpsimd.tensor_add(t12, t1, t2)
                        else:
                            nc.vector.tensor_add(t12, t1, t2)
                        pre_s = epool.tile([128, 512], f16, tag="pre")
                        nc.vector.tensor_add(pre_s, t12, mzims[:, 1024:1536])

                        if nt == 1:  # forget gate -> u1 = sig(f) * c
                            gf = epool.tile([128, 512], f16, tag="gout")
                            nc.scalar.activation(gf, pre_s, ACT.Sigmoid)
                            nc.gpsimd.tensor_mul(
                                u1_all[:, bt, :], gf, mc_all[:, bt, :]
                            )
                        elif nt == 0:  # input gate, keep
                            nc.scalar.activation(
                                g0_all[:, bt, :], pre_s, ACT.Sigmoid
                            )
                        elif nt == 2:  # g gate -> c_new, tanh(c_new)
                            tg = epool.tile([128, 512], f16, tag="gout")
                            nc.scalar.activation(tg, pre_s, ACT.Tanh)
                            u2 = epool.tile([128, 512], f16, tag="u2")
                            nc.vector.tensor_mul(u2, g0_all[:, bt, :], tg)
                            c_new = out_c[:, bt, :]
                            nc.vector.tensor_add(c_new, u2, u1_all[:, bt, :])
                            nc.scalar.activation(
                                tc_all[:, bt, :], c_new, ACT.Tanh
                            )
                        else:  # output gate -> h_new
                            go = epool.tile([128, 512], f16, tag="gout")
                            nc.scalar.activation(go, pre_s, ACT.Sigmoid)
                            nc.gpsimd.tensor_mul(
                                out_h[:, bt, :], go, tc_all[:, bt, :]
                            )
                pspool_cm.__exit__(None, None, None)

                # ============ P4: batched cast stores =====================
                nc.gpsimd.dma_start(
                    mcn_d.rearrange("(c p) f -> p c f", p=128), out_c
                )
                nc.gpsimd.dma_start(
                    mhn_d.rearrange("(c p) f -> p c f", p=128), out_h
                )
                nc.gpsimd.dma_start(
                    zcn_d.rearrange("(c p) f -> p c f", p=128), out_zc
                )
                nc.gpsimd.dma_start(
                    zhn_d.rearrange("(c p) f -> p c f", p=128), out_zh
                )

    nc.finalize()
    return nc


def _get_nc(reps=1, mode="full"):
    key = f"nc{reps}{mode}"
    if key not in _CACHE:
        _CACHE[key] = _build(reps, mode)
    return _CACHE[key]


def kernel(**inputs):
    from concourse import bass_utils

    nc = _get_nc()

    arr = {
        k: np.ascontiguousarray(np.asarray(v, dtype=np.float32))
        for k, v in inputs.items()
    }
    arr["bias_i"] = arr["bias_i"].reshape(E_, 1)
    arr["bias_H"] = arr["bias_H"].reshape(E_, 1)
    arr["bias"] = arr["bias"].reshape(1, G4H)
    arr["bias_hyper"] = arr["bias_hyper"].reshape(1, G4HH)

    in_maps = []
    for c in range(N_CORES):
        m = {}
        for k, v in arr.items():
            m[k] = v[c * B_ : (c + 1) * B_] if k in _BATCH_KEYS else v
        in_maps.append(m)

    res = bass_utils.run_bass_kernel_spmd(
        nc, in_maps, core_ids=list(range(N_CORES)), **_CACHE.get("run_kwargs", {})
    )
    _CACHE["last_results"] = res
    outs = res.results
    main_h_new = np.concatenate([outs[c]["main_h_new"] for c in range(N_CORES)], 0)
    main_c_new = np.concatenate([outs[c]["main_c_new"] for c in range(N_CORES)], 0)
    meta_h_new = np.concatenate([outs[c]["meta_h_new"] for c in range(N_CORES)], 0)
    meta_c_new = np.concatenate([outs[c]["meta_c_new"] for c in range(N_CORES)], 0)
    return (main_h_new, main_c_new, meta_h_new, meta_c_new)
